# revision 20
# baseline (speedup 1.0000x reference)
"""CGCN (multi-head graph-attention block) Trainium2 kernel.

Sharding: 8 cores = (batch b in 0..3) x (query-half in 0..1). Each core
computes, for its 1024 query rows of one batch, the full softmaxed
adjacency slice (8, 1024, 2048) and the output slice (1024, 512).

Inside each core:
  - all matmul operands are bf16 (full-rate PE), accumulation fp32 in PSUM
  - masking is folded into the score matmuls as a 65th contraction row
    (ones row in qa^T, mask-bias row in kb^T)
  - scores are computed twice (q-major for the adjacency output + row
    sums via the activation accumulator; k-major over mask-compacted
    keys for the adj @ support matmul) which avoids PE transposes
  - softmax skips the max-subtraction (scores are O(1); masked entries
    are -1e10 and underflow to exactly 0 in exp)
"""

import math
import numpy as np

B = 4
T = 2048
D = 512
H = 8
DK = 64
Q = 1024  # query rows per core
N_CORES = 8
MASK_WITH = -1.0e10
LN_EPS = 1e-5

_PROG_CACHE = {}


class _act_table_patch:
    """Context manager: during nc.compile(), make every ACT function we use
    resolve to the single set `natural_log_exp_and_others` (contains exp, ln,
    relu, copy, ...) so the kernel never reloads activation tables. Set
    indices are preserved; only membership of the other sets is reduced.
    Restores the original function on exit."""

    def __enter__(self):
        import concourse.hw_specs as hw_specs
        import concourse.bacc as bacc

        self._hw_specs, self._bacc = hw_specs, bacc
        self._orig_hw = hw_specs.get_activation_tables
        self._orig_bacc = bacc.get_activation_tables
        orig = self._orig_hw

        def patched(arch):
            tabs = orig(arch)
            keep = "natural_log_exp_and_others"
            if keep not in tabs:
                return tabs
            target = tabs[keep]
            return {
                name: (fns if name == keep else fns - target)
                for name, fns in tabs.items()
            }

        hw_specs.get_activation_tables = patched
        bacc.get_activation_tables = patched
        return self

    def __exit__(self, *exc):
        self._hw_specs.get_activation_tables = self._orig_hw
        self._bacc.get_activation_tables = self._orig_bacc
        return False


def _build(tcpad):
    import concourse.bacc as bacc
    import concourse.mybir as mybir
    import concourse.tile as tile

    f32 = mybir.dt.float32
    bf16 = mybir.dt.bfloat16
    Exp = mybir.ActivationFunctionType.Exp
    Ln = mybir.ActivationFunctionType.Ln
    Relu = mybir.ActivationFunctionType.Relu
    mult = mybir.AluOpType.mult
    add = mybir.AluOpType.add

    KT = tcpad // 128  # compact k tiles

    nc = bacc.Bacc("TRN2", target_bir_lowering=False)

    qT = nc.declare_dram_parameter("qT", [512, Q], f32, isOutput=False)
    kT = nc.declare_dram_parameter("kT", [512, T], f32, isOutput=False)
    kTc = nc.declare_dram_parameter("kTc", [512, tcpad], f32, isOutput=False)
    qres = nc.declare_dram_parameter("qres", [Q, 512], f32, isOutput=False)
    wqT = nc.declare_dram_parameter("wqT", [512, 512], f32, isOutput=False)
    wkT = nc.declare_dram_parameter("wkT", [512, 512], f32, isOutput=False)
    wgT = nc.declare_dram_parameter("wgT", [512, 512], f32, isOutput=False)
    woT = nc.declare_dram_parameter("woT", [512, 512], f32, isOutput=False)
    mb = nc.declare_dram_parameter("mb", [1, T], f32, isOutput=False)
    mbc = nc.declare_dram_parameter("mbc", [1, tcpad], f32, isOutput=False)
    bo = nc.declare_dram_parameter("bo", [1, 512], f32, isOutput=False)

    adj_p = nc.declare_dram_parameter("adj_p", [H, Q, T], f32, isOutput=True)
    out_p = nc.declare_dram_parameter("out_p", [Q, 512], f32, isOutput=True)

    with tile.TileContext(nc) as tc:
        # ----- persistent pools -----
        with (
            tc.tile_pool(name="acts", bufs=1) as acts,
            tc.tile_pool(name="singles", bufs=1) as singles,
        ):
            # augmented activations: row 64 = ones (qaT) / mask bias (kbT)
            qa = acts.tile([65, H, Q], bf16)      # qa^T / 8 per head
            kb = acts.tile([65, H, T], bf16)      # kb^T per head (full k)
            kbc = acts.tile([65, H, tcpad], bf16)  # kb^T compact k
            sup = acts.tile([128, KT, H, 65], bf16)  # [ones | support dims]
            outT = acts.tile([128, 4, Q], bf16)   # relu'd normalized out^T
            wo_sb = singles.tile([128, 4, 512], bf16)
            bo_sb = singles.tile([1, 512], bf16)
            ones_row = singles.tile([1, 128], bf16)
            eps_t = singles.tile([128, 1], f32)

            nc.gpsimd.dma_start(out=wo_sb, in_=woT.rearrange("(a p) n -> p a n", p=128))
            nc.gpsimd.dma_start(out=bo_sb, in_=bo[:, :])
            nc.vector.memset(ones_row, 1.0)
            nc.vector.memset(eps_t, LN_EPS)
            nc.vector.memset(qa[64:65, :, :], 1.0)
            nc.vector.memset(sup[:, :, :, 64:65], 1.0)
            for h in range(H):
                nc.gpsimd.dma_start(out=kb[64:65, h, :], in_=mb[:, :])
                nc.gpsimd.dma_start(out=kbc[64:65, h, :], in_=mbc[:, :])

            # ----- stage A: input projections -----
            with (
                tc.tile_pool(name="ins", bufs=1) as ins,
                tc.tile_pool(name="psA", bufs=4, space="PSUM") as psA,
            ):
                qT_sb = ins.tile([128, 4, Q], bf16)
                kT_sb = ins.tile([128, 4, T], bf16)
                kTc_sb = ins.tile([128, 4, tcpad], bf16)
                wq_sb = ins.tile([128, 4, 512], bf16, tag="wq")
                wk_sb = ins.tile([128, 4, 512], bf16, tag="wk")
                wg_sb = ins.tile([128, 4, 512], bf16, tag="wg")
                nc.gpsimd.dma_start(out=qT_sb, in_=qT.rearrange("(a p) n -> p a n", p=128))
                nc.gpsimd.dma_start(out=kT_sb, in_=kT.rearrange("(a p) n -> p a n", p=128))
                nc.gpsimd.dma_start(out=kTc_sb, in_=kTc.rearrange("(a p) n -> p a n", p=128))
                nc.gpsimd.dma_start(out=wq_sb, in_=wqT.rearrange("(a p) n -> p a n", p=128))
                nc.gpsimd.dma_start(out=wk_sb, in_=wkT.rearrange("(a p) n -> p a n", p=128))
                nc.gpsimd.dma_start(out=wg_sb, in_=wgT.rearrange("(a p) n -> p a n", p=128))

                # support (+ ones col layout) for compact keys, all heads at once
                for kt in range(KT):
                    ps = psA.tile([128, 512], f32, tag="psA")
                    for dt in range(4):
                        nc.tensor.matmul(
                            ps,
                            kTc_sb[:, dt, kt * 128:(kt + 1) * 128],
                            wg_sb[:, dt, :],
                            start=(dt == 0),
                            stop=(dt == 3),
                        )
                    nc.vector.tensor_copy(
                        out=sup[:, kt, :, 0:64],
                        in_=ps.rearrange("p (h d) -> p h d", h=H),
                    )

                # qa^T (Wq pre-scaled by 1/sqrt(dk) on host)
                def proj_qa_kbc(h):
                    qa = hd.tile([65, Q], bf16, tag="qa")
                    kbc = hd.tile([65, tcpad], bf16, tag="kbc")
                    nc.vector.memset(qa[64:65, :], 1.0)
                    nc.sync.dma_start(out=kbc[64:65, :], in_=mbc[:, :])
                    for qc in range(2):
                        ps = ps_sup.tile([64, 512], f32, tag="psB")
                        for dt in range(4):
                            nc.tensor.matmul(
                                ps,
                                wq_sb[:, dt, h * 64:(h + 1) * 64],
                                qT_sb[:, dt, qc * 512:(qc + 1) * 512],
                                start=(dt == 0),
                                stop=(dt == 3),
                            )
                        nc.vector.tensor_copy(out=qa[0:64, qc * 512:(qc + 1) * 512], in_=ps)
                    for n in range(nkc):
                        w = min(512, tcpad - n * 512)
                        ps = ps_sup.tile([64, 512], f32, tag="psB")
                        for dt in range(4):
                            nc.tensor.matmul(
                                ps[:, 0:w],
                                wk_sb[:, dt, h * 64:(h + 1) * 64],
                                kTc_sb[:, dt, n * 512:n * 512 + w],
                                start=(dt == 0),
                                stop=(dt == 3),
                            )
                        nc.vector.tensor_copy(out=kbc[0:64, n * 512:n * 512 + w], in_=ps[:, 0:w])
                    return qa, kbc

                def proj_kb(h):
                    kb = hd.tile([65, T], bf16, tag="kb")
                    nc.sync.dma_start(out=kb[64:65, :], in_=mb[:, :])
                    for n in range(4):
                        ps = ps_sup.tile([64, 512], f32, tag="psB")
                        for dt in range(4):
                            nc.tensor.matmul(
                                ps,
                                wk_sb[:, dt, h * 64:(h + 1) * 64],
                                kT_sb[:, dt, n * 512:(n + 1) * 512],
                                start=(dt == 0),
                                stop=(dt == 3),
                            )
                        nc.vector.tensor_copy(out=kb[0:64, n * 512:(n + 1) * 512], in_=ps)
                    return kb

                def phase_k(qa, kbc):
                    pt = ptp.tile([128, KT, Q], bf16)
                    for kt in range(KT):
                        ps = ps_st.tile([128, Q], f32)
                        for qc in range(2):
                            nc.tensor.matmul(
                                ps[:, qc * 512:(qc + 1) * 512],
                                kbc[:, kt * 128:(kt + 1) * 128],
                                qa[:, qc * 512:(qc + 1) * 512],
                                start=True,
                                stop=True,
                            )
                        nc.scalar.activation(out=pt[:, kt, :], in_=ps, func=Exp)
                    return pt

                def emit_support():
                    for kt in range(KT):
                        ps = ps_sup.tile([128, 512], f32, tag="psB")
                        for dt in range(4):
                            nc.tensor.matmul(
                                ps,
                                kTc_sb[:, dt, kt * 128:(kt + 1) * 128],
                                wg_sb[:, dt, :],
                                start=(dt == 0),
                                stop=(dt == 3),
                            )
                        nc.vector.tensor_copy(
                            out=sup[:, kt, :, 0:64],
                            in_=ps.rearrange("p (h d) -> p h d", h=H),
                        )

                def emit_q(h, qa, kb):
                    dcol = dnp.tile([128, 8], f32, tag="dcol")
                    rcol = dnp.tile([128, 8], f32, tag="rcol")
                    for qt in range(8):
                        ps = ps_qs.tile([128, T], f32)
                        for n in range(4):
                            nc.tensor.matmul(
                                ps[:, n * 512:(n + 1) * 512],
                                qa[:, qt * 128:(qt + 1) * 128],
                                kb[:, n * 512:(n + 1) * 512],
                                start=True,
                                stop=True,
                            )
                        adj_sb = adjp.tile([128, T], f32)
                        nc.scalar.activation(
                            out=adj_sb, in_=ps, func=Exp,
                            accum_out=dcol[:, qt:qt + 1],
                        )
                        nc.vector.reciprocal(out=rcol[:, qt:qt + 1], in_=dcol[:, qt:qt + 1])
                        nc.vector.tensor_scalar_mul(adj_sb, adj_sb, rcol[:, qt:qt + 1])
                        nc.sync.dma_start(
                            out=adj_p[h, qt * 128:(qt + 1) * 128, :], in_=adj_sb
                        )

                def emit_av(h, pt):
                    j, lo = h // 2, (h % 2) * 64
                    for qc in range(2):
                        av = ps_av.tile([65, 512], f32)
                        for kt in range(KT):
                            nc.tensor.matmul(
                                av,
                                sup[:, kt, h, :],
                                pt[:, kt, qc * 512:(qc + 1) * 512],
                                start=(kt == 0),
                                stop=(kt == KT - 1),
                            )
                        rr = dnp.tile([1, 512], f32, tag="rr")
                        nc.vector.reciprocal(out=rr, in_=av[64:65, :])
                        rep = dnp.tile([64, 512], f32, tag="rep")
                        nc.gpsimd.partition_broadcast(rep, rr)
                        tmp = dnp.tile([64, 512], f32, tag="tmp")
                        nc.vector.tensor_mul(tmp, av[0:64, :], rep)
                        nc.vector.tensor_scalar_max(
                            outT[lo:lo + 64, j, qc * 512:(qc + 1) * 512], tmp, 0.0
                        )

                qa_next, kbc_next = proj_qa_kbc(0)
                for h in range(H):
                    qa, kbc = qa_next, kbc_next
                    pt = phase_k(qa, kbc)
                    kb = proj_kb(h)
                    if h == 0:
                        emit_support()
                    if h + 1 < H:
                        qa_next, kbc_next = proj_qa_kbc(h + 1)
                    emit_q(h, qa, kb)
                    emit_av(h, pt)

            # ----- tail: merge heads, Wo, LN -> relu -> +residual -> LN -----
            with (
                tc.tile_pool(name="tail", bufs=3) as tp,
                tc.tile_pool(name="ps_o", bufs=2, space="PSUM") as ps_o,
            ):
                for qt in range(8):
                    po = ps_o.tile([128, 512], f32)
                    for j in range(4):
                        nc.tensor.matmul(
                            po,
                            outT[:, j, qt * 128:(qt + 1) * 128],
                            wo_sb[:, j, :],
                            start=(j == 0),
                            stop=False,
                        )
                    nc.tensor.matmul(po, ones_row, bo_sb, start=False, stop=True)

                    st = tp.tile([128, 6], f32, tag="st")
                    mv = tp.tile([128, 2], f32, tag="mv")
                    nc.vector.bn_stats(out=st, in_=po)
                    nc.vector.bn_aggr(out=mv, in_=st)
                    lnv = tp.tile([128, 1], f32, tag="lnv")
                    nc.scalar.activation(out=lnv, in_=mv[:, 1:2], func=Ln, bias=eps_t)
                    rstd = tp.tile([128, 1], f32, tag="rstd")
                    nc.scalar.activation(out=rstd, in_=lnv, func=Exp, scale=-0.5)
                    nm = tp.tile([128, 1], f32, tag="nm")
                    nc.vector.tensor_mul(nm, mv[:, 0:1], rstd)
                    nc.vector.tensor_scalar_mul(nm, nm, -1.0)
                    t1 = tp.tile([128, 512], f32, tag="t1")
                    nc.scalar.activation(out=t1, in_=po, func=Relu, scale=rstd, bias=nm)

                    qr_t = tp.tile([128, 512], f32, tag="qr")
                    nc.sync.dma_start(out=qr_t, in_=qres[qt * 128:(qt + 1) * 128, :])
                    t2 = tp.tile([128, 512], f32, tag="t2")
                    nc.vector.tensor_add(t2, t1, qr_t)

                    st2 = tp.tile([128, 6], f32, tag="st2")
                    mv2 = tp.tile([128, 2], f32, tag="mv2")
                    nc.vector.bn_stats(out=st2, in_=t2)
                    nc.vector.bn_aggr(out=mv2, in_=st2)
                    lnv2 = tp.tile([128, 1], f32, tag="lnv2")
                    nc.scalar.activation(out=lnv2, in_=mv2[:, 1:2], func=Ln, bias=eps_t)
                    rstd2 = tp.tile([128, 1], f32, tag="rstd2")
                    nc.scalar.activation(out=rstd2, in_=lnv2, func=Exp, scale=-0.5)
                    nm2 = tp.tile([128, 1], f32, tag="nm2")
                    nc.vector.tensor_mul(nm2, mv2[:, 0:1], rstd2)
                    nc.vector.tensor_scalar_mul(nm2, nm2, -1.0)
                    o_sb = tp.tile([128, 512], f32, tag="osb")
                    nc.vector.tensor_scalar(o_sb, t2, rstd2, nm2, mult, add)
                    nc.sync.dma_start(out=out_p[qt * 128:(qt + 1) * 128, :], in_=o_sb)

    with _act_table_patch():
        nc.compile()
    return nc


def _prep(query, key, mask, Wq, Wk, Wg, Wo, bo):
    query = np.asarray(query, dtype=np.float32)
    key = np.asarray(key, dtype=np.float32)
    mask = np.asarray(mask)
    Wq = np.asarray(Wq, dtype=np.float32)
    Wk = np.asarray(Wk, dtype=np.float32)
    Wg = np.asarray(Wg, dtype=np.float32)
    Wo = np.asarray(Wo, dtype=np.float32)
    bo = np.asarray(bo, dtype=np.float32)

    m01 = (mask[:, 0, :] != 0)  # (B, T) bool
    sels = [np.nonzero(m01[b])[0] for b in range(B)]
    tcs = [len(s) for s in sels]
    tcpad = max(128, ((max(tcs) + 127) // 128) * 128)

    wqT = np.ascontiguousarray(Wq.T / math.sqrt(DK))
    wkT = np.ascontiguousarray(Wk.T)
    wgT = np.ascontiguousarray(Wg.T)
    woT = np.ascontiguousarray(Wo.T)
    bo2 = np.ascontiguousarray(bo.reshape(1, 512))

    in_maps = []
    for c in range(N_CORES):
        b, qh = c // 2, c % 2
        kt_full = np.ascontiguousarray(key[b].T)  # (512, T)
        ktc = np.zeros((512, tcpad), dtype=np.float32)
        ktc[:, : tcs[b]] = kt_full[:, sels[b]]
        mb = np.where(m01[b], 0.0, MASK_WITH).astype(np.float32).reshape(1, T)
        mbc = np.full((1, tcpad), MASK_WITH, dtype=np.float32)
        mbc[0, : tcs[b]] = 0.0
        in_maps.append(
            {
                "qT": np.ascontiguousarray(query[b, qh * Q:(qh + 1) * Q, :].T),
                "kT": kt_full,
                "kTc": ktc,
                "qres": np.ascontiguousarray(query[b, qh * Q:(qh + 1) * Q, :]),
                "wqT": wqT,
                "wkT": wkT,
                "wgT": wgT,
                "woT": woT,
                "mb": mb,
                "mbc": mbc,
                "bo": bo2,
            }
        )
    return in_maps, tcpad


def _run(query, key, mask, Wq, Wk, Wg, Wo, bo, trace=False):
    from concourse.bass_utils import run_bass_kernel_spmd

    in_maps, tcpad = _prep(query, key, mask, Wq, Wk, Wg, Wo, bo)
    if tcpad not in _PROG_CACHE:
        _PROG_CACHE[tcpad] = _build(tcpad)
    nc = _PROG_CACHE[tcpad]
    res = run_bass_kernel_spmd(
        nc, in_maps, list(range(N_CORES)), trace=trace,
    )

    out = np.empty((B, T, D), dtype=np.float32)
    adj = np.empty((B, H, T, T), dtype=np.float32)
    for c in range(N_CORES):
        b, qh = c // 2, c % 2
        out[b, qh * Q:(qh + 1) * Q, :] = res.results[c]["out_p"]
        adj[b, :, qh * Q:(qh + 1) * Q, :] = res.results[c]["adj_p"]
    return (out, adj), res


def kernel(query, key, mask, Wq, Wk, Wg, Wo, bo):
    (out, adj), _ = _run(query, key, mask, Wq, Wk, Wg, Wo, bo, trace=False)
    return (out, adj)


# revision 22
# speedup vs baseline: 299.9363x; 299.9363x over previous
"""CGCN (multi-head graph-attention block) Trainium2 kernel.

Sharding: 8 cores = (batch b in 0..3) x (query-half in 0..1). Each core
computes, for its 1024 query rows of one batch, the full softmaxed
adjacency slice (8, 1024, 2048) and the output slice (1024, 512).

Inside each core:
  - all matmul operands are bf16 (full-rate PE), accumulation fp32 in PSUM
  - masking is folded into the score matmuls as a 65th contraction row
    (ones row in qa^T, mask-bias row in kb^T)
  - scores are computed twice (q-major for the adjacency output + row
    sums via the activation accumulator; k-major over mask-compacted
    keys for the adj @ support matmul) which avoids PE transposes
  - softmax skips the max-subtraction (scores are O(1); masked entries
    are -1e10 and underflow to exactly 0 in exp)
"""

import math
import numpy as np

B = 4
T = 2048
D = 512
H = 8
DK = 64
Q = 1024  # query rows per core
N_CORES = 8
MASK_WITH = -1.0e10
LN_EPS = 1e-5

_PROG_CACHE = {}


class _act_table_patch:
    """Context manager: during nc.compile(), make every ACT function we use
    resolve to the single set `natural_log_exp_and_others` (contains exp, ln,
    relu, copy, ...) so the kernel never reloads activation tables. Set
    indices are preserved; only membership of the other sets is reduced.
    Restores the original function on exit."""

    def __enter__(self):
        import concourse.hw_specs as hw_specs
        import concourse.bacc as bacc

        self._hw_specs, self._bacc = hw_specs, bacc
        self._orig_hw = hw_specs.get_activation_tables
        self._orig_bacc = bacc.get_activation_tables
        orig = self._orig_hw

        def patched(arch):
            tabs = orig(arch)
            keep = "natural_log_exp_and_others"
            if keep not in tabs:
                return tabs
            target = tabs[keep]
            return {
                name: (fns if name == keep else fns - target)
                for name, fns in tabs.items()
            }

        hw_specs.get_activation_tables = patched
        bacc.get_activation_tables = patched
        return self

    def __exit__(self, *exc):
        self._hw_specs.get_activation_tables = self._orig_hw
        self._bacc.get_activation_tables = self._orig_bacc
        return False


def _build(tcpad):
    import concourse.bacc as bacc
    import concourse.mybir as mybir
    import concourse.tile as tile

    f32 = mybir.dt.float32
    bf16 = mybir.dt.bfloat16
    Exp = mybir.ActivationFunctionType.Exp
    Ln = mybir.ActivationFunctionType.Ln
    Relu = mybir.ActivationFunctionType.Relu
    mult = mybir.AluOpType.mult
    add = mybir.AluOpType.add

    KT = tcpad // 128  # compact k tiles

    nc = bacc.Bacc("TRN2", target_bir_lowering=False)

    qT = nc.declare_dram_parameter("qT", [512, Q], f32, isOutput=False)
    kT = nc.declare_dram_parameter("kT", [512, T], f32, isOutput=False)
    kTc = nc.declare_dram_parameter("kTc", [512, tcpad], f32, isOutput=False)
    qres = nc.declare_dram_parameter("qres", [Q, 512], f32, isOutput=False)
    wqT = nc.declare_dram_parameter("wqT", [512, 512], f32, isOutput=False)
    wkT = nc.declare_dram_parameter("wkT", [512, 512], f32, isOutput=False)
    wgT = nc.declare_dram_parameter("wgT", [512, 512], f32, isOutput=False)
    woT = nc.declare_dram_parameter("woT", [512, 512], f32, isOutput=False)
    mb = nc.declare_dram_parameter("mb", [1, T], f32, isOutput=False)
    mbc = nc.declare_dram_parameter("mbc", [1, tcpad], f32, isOutput=False)
    bo = nc.declare_dram_parameter("bo", [1, 512], f32, isOutput=False)

    adj_p = nc.declare_dram_parameter("adj_p", [H, Q, T], f32, isOutput=True)
    out_p = nc.declare_dram_parameter("out_p", [Q, 512], f32, isOutput=True)

    with tile.TileContext(nc) as tc:
        # ----- persistent pools -----
        with (
            tc.tile_pool(name="acts", bufs=1) as acts,
            tc.tile_pool(name="singles", bufs=1) as singles,
        ):
            # augmented activations: row 64 = ones (qaT) / mask bias (kbT)
            qa = acts.tile([65, H, Q], bf16)      # qa^T / 8 per head
            kb = acts.tile([65, H, T], bf16)      # kb^T per head (full k)
            kbc = acts.tile([65, H, tcpad], bf16)  # kb^T compact k
            sup = acts.tile([128, KT, H, 65], bf16)  # [ones | support dims]
            outT = acts.tile([128, 4, Q], bf16)   # relu'd normalized out^T
            wo_sb = singles.tile([128, 4, 512], bf16)
            bo_sb = singles.tile([1, 512], bf16)
            ones_row = singles.tile([1, 128], bf16)
            eps_t = singles.tile([128, 1], f32)

            nc.gpsimd.dma_start(out=wo_sb, in_=woT.rearrange("(a p) n -> p a n", p=128))
            nc.gpsimd.dma_start(out=bo_sb, in_=bo[:, :])
            nc.vector.memset(ones_row, 1.0)
            nc.vector.memset(eps_t, LN_EPS)
            nc.vector.memset(qa[64:65, :, :], 1.0)
            nc.vector.memset(sup[:, :, :, 64:65], 1.0)
            for h in range(H):
                nc.gpsimd.dma_start(out=kb[64:65, h, :], in_=mb[:, :])
                nc.gpsimd.dma_start(out=kbc[64:65, h, :], in_=mbc[:, :])

            # ----- stage A: input projections -----
            with (
                tc.tile_pool(name="ins", bufs=1) as ins,
                tc.tile_pool(name="psA", bufs=4, space="PSUM") as psA,
            ):
                qT_sb = ins.tile([128, 4, Q], bf16)
                kT_sb = ins.tile([128, 4, T], bf16)
                kTc_sb = ins.tile([128, 4, tcpad], bf16)
                wq_sb = ins.tile([128, 4, 512], bf16, tag="wq")
                wk_sb = ins.tile([128, 4, 512], bf16, tag="wk")
                wg_sb = ins.tile([128, 4, 512], bf16, tag="wg")
                nc.gpsimd.dma_start(out=qT_sb, in_=qT.rearrange("(a p) n -> p a n", p=128))
                nc.gpsimd.dma_start(out=kT_sb, in_=kT.rearrange("(a p) n -> p a n", p=128))
                nc.gpsimd.dma_start(out=kTc_sb, in_=kTc.rearrange("(a p) n -> p a n", p=128))
                nc.gpsimd.dma_start(out=wq_sb, in_=wqT.rearrange("(a p) n -> p a n", p=128))
                nc.gpsimd.dma_start(out=wk_sb, in_=wkT.rearrange("(a p) n -> p a n", p=128))
                nc.gpsimd.dma_start(out=wg_sb, in_=wgT.rearrange("(a p) n -> p a n", p=128))

                # support (+ ones col layout) for compact keys, all heads at once
                for kt in range(KT):
                    ps = psA.tile([128, 512], f32, tag="psA")
                    for dt in range(4):
                        nc.tensor.matmul(
                            ps,
                            kTc_sb[:, dt, kt * 128:(kt + 1) * 128],
                            wg_sb[:, dt, :],
                            start=(dt == 0),
                            stop=(dt == 3),
                        )
                    nc.vector.tensor_copy(
                        out=sup[:, kt, :, 0:64],
                        in_=ps.rearrange("p (h d) -> p h d", h=H),
                    )

                # qa^T (Wq pre-scaled by 1/sqrt(dk) on host)
                def proj_qa_kbc(h):
                    qa = hd.tile([65, Q], bf16, tag="qa")
                    kbc = hd.tile([65, tcpad], bf16, tag="kbc")
                    nc.vector.memset(qa[64:65, :], 1.0)
                    nc.sync.dma_start(out=kbc[64:65, :], in_=mbc[:, :])
                    for qc in range(2):
                        ps = ps_sup.tile([64, 512], f32, tag="psB")
                        for dt in range(4):
                            nc.tensor.matmul(
                                ps,
                                wq_sb[:, dt, h * 64:(h + 1) * 64],
                                qT_sb[:, dt, qc * 512:(qc + 1) * 512],
                                start=(dt == 0),
                                stop=(dt == 3),
                            )
                        nc.vector.tensor_copy(out=qa[0:64, qc * 512:(qc + 1) * 512], in_=ps)
                    for n in range(nkc):
                        w = min(512, tcpad - n * 512)
                        ps = ps_sup.tile([64, 512], f32, tag="psB")
                        for dt in range(4):
                            nc.tensor.matmul(
                                ps[:, 0:w],
                                wk_sb[:, dt, h * 64:(h + 1) * 64],
                                kTc_sb[:, dt, n * 512:n * 512 + w],
                                start=(dt == 0),
                                stop=(dt == 3),
                            )
                        nc.vector.tensor_copy(out=kbc[0:64, n * 512:n * 512 + w], in_=ps[:, 0:w])
                    return qa, kbc

                def proj_kb(h):
                    kb = hd.tile([65, T], bf16, tag="kb")
                    nc.sync.dma_start(out=kb[64:65, :], in_=mb[:, :])
                    for n in range(4):
                        ps = ps_sup.tile([64, 512], f32, tag="psB")
                        for dt in range(4):
                            nc.tensor.matmul(
                                ps,
                                wk_sb[:, dt, h * 64:(h + 1) * 64],
                                kT_sb[:, dt, n * 512:(n + 1) * 512],
                                start=(dt == 0),
                                stop=(dt == 3),
                            )
                        nc.vector.tensor_copy(out=kb[0:64, n * 512:(n + 1) * 512], in_=ps)
                    return kb

                def phase_k(qa, kbc):
                    pt = ptp.tile([128, KT, Q], bf16)
                    for kt in range(KT):
                        ps = ps_st.tile([128, Q], f32)
                        for qc in range(2):
                            nc.tensor.matmul(
                                ps[:, qc * 512:(qc + 1) * 512],
                                kbc[:, kt * 128:(kt + 1) * 128],
                                qa[:, qc * 512:(qc + 1) * 512],
                                start=True,
                                stop=True,
                            )
                        nc.scalar.activation(out=pt[:, kt, :], in_=ps, func=Exp)
                    return pt

                def emit_support():
                    for kt in range(KT):
                        ps = ps_sup.tile([128, 512], f32, tag="psB")
                        for dt in range(4):
                            nc.tensor.matmul(
                                ps,
                                kTc_sb[:, dt, kt * 128:(kt + 1) * 128],
                                wg_sb[:, dt, :],
                                start=(dt == 0),
                                stop=(dt == 3),
                            )
                        nc.vector.tensor_copy(
                            out=sup[:, kt, :, 0:64],
                            in_=ps.rearrange("p (h d) -> p h d", h=H),
                        )

                def emit_q(h, qa, kb):
                    dcol = dnp.tile([128, 8], f32, tag="dcol")
                    rcol = dnp.tile([128, 8], f32, tag="rcol")
                    for qt in range(8):
                        ps = ps_qs.tile([128, T], f32)
                        for n in range(4):
                            nc.tensor.matmul(
                                ps[:, n * 512:(n + 1) * 512],
                                qa[:, qt * 128:(qt + 1) * 128],
                                kb[:, n * 512:(n + 1) * 512],
                                start=True,
                                stop=True,
                            )
                        adj_sb = adjp.tile([128, T], f32)
                        nc.scalar.activation(
                            out=adj_sb, in_=ps, func=Exp,
                            accum_out=dcol[:, qt:qt + 1],
                        )
                        nc.vector.reciprocal(out=rcol[:, qt:qt + 1], in_=dcol[:, qt:qt + 1])
                        nc.vector.tensor_scalar_mul(adj_sb, adj_sb, rcol[:, qt:qt + 1])
                        nc.sync.dma_start(
                            out=adj_p[h, qt * 128:(qt + 1) * 128, :], in_=adj_sb
                        )

                def emit_av(h, pt):
                    j, lo = h // 2, (h % 2) * 64
                    for qc in range(2):
                        av = ps_av.tile([65, 512], f32)
                        for kt in range(KT):
                            nc.tensor.matmul(
                                av,
                                sup[:, kt, h, :],
                                pt[:, kt, qc * 512:(qc + 1) * 512],
                                start=(kt == 0),
                                stop=(kt == KT - 1),
                            )
                        rr = dnp.tile([1, 512], f32, tag="rr")
                        nc.vector.reciprocal(out=rr, in_=av[64:65, :])
                        rep = dnp.tile([64, 512], f32, tag="rep")
                        nc.gpsimd.partition_broadcast(rep, rr)
                        tmp = dnp.tile([64, 512], f32, tag="tmp")
                        nc.vector.tensor_mul(tmp, av[0:64, :], rep)
                        nc.vector.tensor_scalar_max(
                            outT[lo:lo + 64, j, qc * 512:(qc + 1) * 512], tmp, 0.0
                        )

                qa_next, kbc_next = proj_qa_kbc(0)
                for h in range(H):
                    qa, kbc = qa_next, kbc_next
                    pt = phase_k(qa, kbc)
                    kb = proj_kb(h)
                    if h == 0:
                        emit_support()
                    if h + 1 < H:
                        qa_next, kbc_next = proj_qa_kbc(h + 1)
                    emit_q(h, qa, kb)
                    emit_av(h, pt)

            # ----- tail: merge heads, Wo, LN -> relu -> +residual -> LN -----
            with (
                tc.tile_pool(name="tail", bufs=3) as tp,
                tc.tile_pool(name="ps_o", bufs=2, space="PSUM") as ps_o,
            ):
                for qt in range(8):
                    po = ps_o.tile([128, 512], f32)
                    for j in range(4):
                        nc.tensor.matmul(
                            po,
                            outT[:, j, qt * 128:(qt + 1) * 128],
                            wo_sb[:, j, :],
                            start=(j == 0),
                            stop=False,
                        )
                    nc.tensor.matmul(po, ones_row, bo_sb, start=False, stop=True)

                    st = tp.tile([128, 6], f32, tag="st")
                    mv = tp.tile([128, 2], f32, tag="mv")
                    nc.vector.bn_stats(out=st, in_=po)
                    nc.vector.bn_aggr(out=mv, in_=st)
                    lnv = tp.tile([128, 1], f32, tag="lnv")
                    nc.scalar.activation(out=lnv, in_=mv[:, 1:2], func=Ln, bias=eps_t)
                    rstd = tp.tile([128, 1], f32, tag="rstd")
                    nc.scalar.activation(out=rstd, in_=lnv, func=Exp, scale=-0.5)
                    nm = tp.tile([128, 1], f32, tag="nm")
                    nc.vector.tensor_mul(nm, mv[:, 0:1], rstd)
                    nc.vector.tensor_scalar_mul(nm, nm, -1.0)
                    t1 = tp.tile([128, 512], f32, tag="t1")
                    nc.scalar.activation(out=t1, in_=po, func=Relu, scale=rstd, bias=nm)

                    qr_t = tp.tile([128, 512], f32, tag="qr")
                    nc.sync.dma_start(out=qr_t, in_=qres[qt * 128:(qt + 1) * 128, :])
                    t2 = tp.tile([128, 512], f32, tag="t2")
                    nc.vector.tensor_add(t2, t1, qr_t)

                    st2 = tp.tile([128, 6], f32, tag="st2")
                    mv2 = tp.tile([128, 2], f32, tag="mv2")
                    nc.vector.bn_stats(out=st2, in_=t2)
                    nc.vector.bn_aggr(out=mv2, in_=st2)
                    lnv2 = tp.tile([128, 1], f32, tag="lnv2")
                    nc.scalar.activation(out=lnv2, in_=mv2[:, 1:2], func=Ln, bias=eps_t)
                    rstd2 = tp.tile([128, 1], f32, tag="rstd2")
                    nc.scalar.activation(out=rstd2, in_=lnv2, func=Exp, scale=-0.5)
                    nm2 = tp.tile([128, 1], f32, tag="nm2")
                    nc.vector.tensor_mul(nm2, mv2[:, 0:1], rstd2)
                    nc.vector.tensor_scalar_mul(nm2, nm2, -1.0)
                    o_sb = tp.tile([128, 512], f32, tag="osb")
                    nc.vector.tensor_scalar(o_sb, t2, rstd2, nm2, mult, add)
                    nc.sync.dma_start(out=out_p[qt * 128:(qt + 1) * 128, :], in_=o_sb)

    with _act_table_patch():
        nc.compile()
    return nc


def _prep(query, key, mask, Wq, Wk, Wg, Wo, bo):
    query = np.asarray(query, dtype=np.float32)
    key = np.asarray(key, dtype=np.float32)
    mask = np.asarray(mask)
    Wq = np.asarray(Wq, dtype=np.float32)
    Wk = np.asarray(Wk, dtype=np.float32)
    Wg = np.asarray(Wg, dtype=np.float32)
    Wo = np.asarray(Wo, dtype=np.float32)
    bo = np.asarray(bo, dtype=np.float32)

    m01 = (mask[:, 0, :] != 0)  # (B, T) bool
    sels = [np.nonzero(m01[b])[0] for b in range(B)]
    tcs = [len(s) for s in sels]
    tcpad = max(128, ((max(tcs) + 127) // 128) * 128)

    wqT = np.ascontiguousarray(Wq.T / math.sqrt(DK))
    wkT = np.ascontiguousarray(Wk.T)
    wgT = np.ascontiguousarray(Wg.T)
    woT = np.ascontiguousarray(Wo.T)
    bo2 = np.ascontiguousarray(bo.reshape(1, 512))

    in_maps = []
    for c in range(N_CORES):
        b, qh = c // 2, c % 2
        kt_full = np.ascontiguousarray(key[b].T)  # (512, T)
        ktc = np.zeros((512, tcpad), dtype=np.float32)
        ktc[:, : tcs[b]] = kt_full[:, sels[b]]
        mb = np.where(m01[b], 0.0, MASK_WITH).astype(np.float32).reshape(1, T)
        mbc = np.full((1, tcpad), MASK_WITH, dtype=np.float32)
        mbc[0, : tcs[b]] = 0.0
        in_maps.append(
            {
                "qT": np.ascontiguousarray(query[b, qh * Q:(qh + 1) * Q, :].T),
                "kT": kt_full,
                "kTc": ktc,
                "qres": np.ascontiguousarray(query[b, qh * Q:(qh + 1) * Q, :]),
                "wqT": wqT,
                "wkT": wkT,
                "wgT": wgT,
                "woT": woT,
                "mb": mb,
                "mbc": mbc,
                "bo": bo2,
            }
        )
    return in_maps, tcpad


def _run(query, key, mask, Wq, Wk, Wg, Wo, bo, trace=False):
    from concourse.bass_utils import run_bass_kernel_spmd

    in_maps, tcpad = _prep(query, key, mask, Wq, Wk, Wg, Wo, bo)
    if tcpad not in _PROG_CACHE:
        _PROG_CACHE[tcpad] = _build(tcpad)
    nc = _PROG_CACHE[tcpad]
    res = run_bass_kernel_spmd(
        nc, in_maps, list(range(N_CORES)), trace=trace,
    )

    out = np.empty((B, T, D), dtype=np.float32)
    adj = np.empty((B, H, T, T), dtype=np.float32)
    for c in range(N_CORES):
        b, qh = c // 2, c % 2
        out[b, qh * Q:(qh + 1) * Q, :] = res.results[c]["out_p"]
        adj[b, :, qh * Q:(qh + 1) * Q, :] = res.results[c]["adj_p"]
    return (out, adj), res


def kernel(query, key, mask, Wq, Wk, Wg, Wo, bo):
    (out, adj), _ = _run(query, key, mask, Wq, Wk, Wg, Wo, bo, trace=False)
    return (out, adj)
